# revision 3
# baseline (speedup 1.0000x reference)
"""Bahdanau attention Trainium2 Bass kernel.

Problem: B=16, S=4096, H=1024, keys dim 2H=2048.
  q_proj = query @ Wq_w^T-ish (einsum bqh,oh->bqo) + Wq_b          [B,1,2048]
  k_proj = keys @ Wk_w^T (einsum bsk,ok->bso) + Wk_b               [B,S,2048]
  scores = tanh(q_proj + k_proj) @ v_w^T + v_b                     [B,S,1]
  attn   = softmax(scores, axis=S)                                 [B,S,1]
  ctx    = attn^T @ keys                                           [B,1,2048]
(v_b cancels in softmax - never used.)

Sharding: data-parallel over batch, 2 batches per core on 8 cores.

Per-core layout strategy (all fp32):
  - Wk^T [k,o] built once in SBUF via PE transposes (16 tiles [128,2048]).
  - Per 256-row s-chunk: keys rows DMA'd naturally, PE-transposed to
    keysT [k, s] tiles, matmul'd against WkT into k_projT [o, s-chunk]
    PSUM, ACT applies tanh with per-partition bias (q_proj + Wk_b),
    PE matvec with v gives scores [1, s-chunk], ACT exp (no max
    subtraction needed: |scores| <= ~36 so fp32 exp is safe) with
    accumulated denominator, PE K=1-transposes exp row into columns,
    and context accumulates via PE matmul against the still-resident
    natural keys tiles. Single pass over keys.
"""

import numpy as np

B, S, H, O = 16, 4096, 1024, 2048  # O = 2*H = keys feature dim
NCORES = 8
BPC = B // NCORES  # batches per core
P = 128
SC = 256  # s-chunk size
F32 = np.float32

_CACHE = {}


def _build(bpc, s_total):
    import concourse.bass as bass
    import concourse.tile as tile
    from concourse import bacc, mybir
    from concourse.masks import make_identity

    dt = mybir.dt.float32
    AF = mybir.ActivationFunctionType
    n_chunks = s_total // SC
    KT = O // P   # 16 k-tiles (keys feature dim)
    OT = O // P   # 16 o-tiles (projection dim)
    HT = H // P   # 8 h-tiles (query dim)

    nc = bacc.Bacc("TRN2", target_bir_lowering=False, debug=False)

    q_in = nc.dram_tensor("query", [bpc, H], dt, kind="ExternalInput")
    k_in = nc.dram_tensor("keys", [bpc, s_total, O], dt, kind="ExternalInput")
    wq_in = nc.dram_tensor("wq", [O, H], dt, kind="ExternalInput")
    wqb_in = nc.dram_tensor("wqb", [O], dt, kind="ExternalInput")
    wk_in = nc.dram_tensor("wk", [O, O], dt, kind="ExternalInput")
    wkb_in = nc.dram_tensor("wkb", [O], dt, kind="ExternalInput")
    v_in = nc.dram_tensor("vw", [1, O], dt, kind="ExternalInput")
    ctx_out = nc.dram_tensor("ctx_out", [bpc, O], dt, kind="ExternalOutput")
    attn_out = nc.dram_tensor(
        "attn_out", [bpc, P, s_total // P], dt, kind="ExternalOutput"
    )

    with tile.TileContext(nc) as tc:
        with (
            tc.tile_pool(name="wkt", bufs=1) as wkt_pool,
            tc.tile_pool(name="knat", bufs=4) as knat_pool,
            tc.tile_pool(name="misc", bufs=1) as misc,
            tc.tile_pool(name="ps_tr", bufs=2, space="PSUM") as ps_tr,
            tc.tile_pool(name="ps_mm", bufs=2, space="PSUM") as ps_mm,
            tc.tile_pool(name="ps_row", bufs=2, space="PSUM") as ps_row,
            tc.tile_pool(name="ps_ctx", bufs=2, space="PSUM") as ps_ctx,
        ):
            ident = misc.tile([P, P], dt, tag="ident")
            make_identity(nc, ident)
            ones1 = misc.tile([1, 1], dt, tag="ones1")
            nc.gpsimd.memset(ones1, 1.0)
            ones_row = misc.tile([1, P], dt, tag="ones_row")
            nc.gpsimd.memset(ones_row, 1.0)

            # ---- init phase: weights transposes + per-batch bias columns ----
            vcol = misc.tile([P, OT], dt, tag="vcol")
            qcols = [misc.tile([P, HT], dt, tag=f"qcols{b}", name=f"qcols{b}") for b in range(bpc)]
            biascol = [misc.tile([P, OT], dt, tag=f"biascol{b}", name=f"biascol{b}") for b in range(bpc)]

            with tc.tile_pool(name="rows", bufs=2) as rows_pool, tc.tile_pool(
                name="wblk", bufs=3
            ) as wblk_pool:
                # v as columns: v[m*128+p] -> vcol[p, m]
                vrow = rows_pool.tile([1, O], dt, tag="row")
                nc.sync.dma_start(out=vrow, in_=v_in[0:1, :])
                ps_v = ps_row.tile([P, OT], dt, tag="row")
                for m in range(OT):
                    nc.tensor.matmul(
                        ps_v[:, m : m + 1], vrow[0:1, m * P : (m + 1) * P], ones1,
                        start=True, stop=True,
                    )
                nc.vector.tensor_copy(vcol, ps_v)

                # query as columns per batch
                for b in range(bpc):
                    qrow = rows_pool.tile([1, H], dt, tag="row")
                    nc.sync.dma_start(out=qrow, in_=q_in[b : b + 1, :])
                    ps_q = ps_row.tile([P, HT], dt, tag="row")
                    for i in range(HT):
                        nc.tensor.matmul(
                            ps_q[:, i : i + 1], qrow[0:1, i * P : (i + 1) * P], ones1,
                            start=True, stop=True,
                        )
                    nc.vector.tensor_copy(qcols[b], ps_q)

                wqb_row = rows_pool.tile([1, O], dt, tag="row")
                nc.sync.dma_start(out=wqb_row, in_=wqb_in[None, :])
                wkb_row = rows_pool.tile([1, O], dt, tag="row")
                nc.sync.dma_start(out=wkb_row, in_=wkb_in[None, :])

                # bias columns: biascol[b][:, j] = (Wq @ query_b + Wq_b + Wk_b)[o-tile j]
                ps_bias = [ps_mm.tile([P, OT], dt, tag="mm", name=f"ps_bias{b}") for b in range(bpc)]
                for j in range(OT):
                    wq_nat = knat_pool.tile([P, H], dt, tag="knat")
                    nc.sync.dma_start(out=wq_nat, in_=wq_in[j * P : (j + 1) * P, :])
                    for i in range(HT):
                        ps_w = ps_tr.tile([P, P], dt, tag="tr")
                        nc.tensor.transpose(
                            ps_w, wq_nat[:, i * P : (i + 1) * P], ident
                        )
                        wqt_blk = wblk_pool.tile([P, P], dt, tag="wblk")
                        nc.vector.tensor_copy(wqt_blk, ps_w)
                        for b in range(bpc):
                            nc.tensor.matmul(
                                ps_bias[b][:, j : j + 1],
                                wqt_blk,
                                qcols[b][:, i : i + 1],
                                start=(i == 0), stop=False,
                            )
                    for b in range(bpc):
                        nc.tensor.matmul(
                            ps_bias[b][:, j : j + 1],
                            wqb_row[0:1, j * P : (j + 1) * P],
                            ones1,
                            start=False, stop=False,
                        )
                        nc.tensor.matmul(
                            ps_bias[b][:, j : j + 1],
                            wkb_row[0:1, j * P : (j + 1) * P],
                            ones1,
                            start=False, stop=True,
                        )
                for b in range(bpc):
                    nc.vector.tensor_copy(biascol[b], ps_bias[b])

                # WkT resident tiles: wkt[kk][p, :] holds Wk[:, kk*128+p]
                wkt = [wkt_pool.tile([P, O], dt, tag=f"wkt{kk}", name=f"wkt{kk}") for kk in range(KT)]
                for j in range(OT):
                    wk_nat = knat_pool.tile([P, O], dt, tag="knat")
                    nc.sync.dma_start(out=wk_nat, in_=wk_in[j * P : (j + 1) * P, :])
                    for kk in range(KT):
                        ps_w = ps_tr.tile([P, P], dt, tag="tr")
                        nc.tensor.transpose(
                            ps_w, wk_nat[:, kk * P : (kk + 1) * P], ident
                        )
                        nc.vector.tensor_copy(
                            wkt[kk][:, j * P : (j + 1) * P], ps_w
                        )

            # ---- main loop ----
            with (
                tc.tile_pool(name="ktr", bufs=KT) as ktr_pool,
                tc.tile_pool(name="tt", bufs=OT) as tt_pool,
                tc.tile_pool(name="prow", bufs=2) as prow_pool,
                tc.tile_pool(name="ctxp", bufs=1) as ctxp,
                tc.tile_pool(name="attnp", bufs=1) as attnp,
            ):
                for b in range(bpc):
                    ctxacc = ctxp.tile([1, O], dt, tag="ctxacc")
                    nc.gpsimd.memset(ctxacc, 0.0)
                    attn_cols = attnp.tile([P, s_total // P], dt, tag=f"ac{b}")
                    denp = misc.tile([1, n_chunks], dt, tag=f"denp{b}")

                    for c in range(n_chunks):
                        knat = []
                        for jj in range(SC // P):
                            kn = knat_pool.tile([P, O], dt, tag="knat", name="kn")
                            s0 = c * SC + jj * P
                            nc.sync.dma_start(out=kn, in_=k_in[b, s0 : s0 + P, :])
                            knat.append(kn)

                        # transpose keys chunk -> keysT [k, s-chunk]
                        ktr = []
                        for kk in range(KT):
                            ps_t = ps_tr.tile([P, SC], dt, tag="tr")
                            for jj in range(SC // P):
                                nc.tensor.transpose(
                                    ps_t[:, jj * P : (jj + 1) * P],
                                    knat[jj][:, kk * P : (kk + 1) * P],
                                    ident,
                                )
                            kt = ktr_pool.tile([P, SC], dt, tag="ktr")
                            nc.vector.tensor_copy(kt, ps_t)
                            ktr.append(kt)

                        # k_projT [o-tile, s-chunk] + tanh
                        tts = []
                        for m in range(OT):
                            ps_k = ps_mm.tile([P, SC], dt, tag="mm")
                            for kk in range(KT):
                                nc.tensor.matmul(
                                    ps_k,
                                    wkt[kk][:, m * P : (m + 1) * P],
                                    ktr[kk],
                                    start=(kk == 0), stop=(kk == KT - 1),
                                )
                            t_m = tt_pool.tile([P, SC], dt, tag="tt")
                            nc.scalar.activation(
                                t_m, ps_k, AF.Tanh, bias=biascol[b][:, m : m + 1]
                            )
                            tts.append(t_m)

                        # scores [1, s-chunk]
                        ps_sc = ps_row.tile([1, SC], dt, tag="row")
                        for m in range(OT):
                            nc.tensor.matmul(
                                ps_sc, vcol[:, m : m + 1], tts[m],
                                start=(m == 0), stop=(m == OT - 1),
                            )

                        # p = exp(scores); denominator partial
                        prow = prow_pool.tile([1, SC], dt, tag="prow")
                        nc.scalar.activation(
                            prow, ps_sc, AF.Exp, accum_out=denp[:, c : c + 1]
                        )

                        # p row -> columns, into attn_cols
                        ps_pt = ps_row.tile([P, SC // P], dt, tag="row")
                        for jj in range(SC // P):
                            nc.tensor.matmul(
                                ps_pt[:, jj : jj + 1],
                                prow[0:1, jj * P : (jj + 1) * P],
                                ones1,
                                start=True, stop=True,
                            )
                        ncc = SC // P
                        nc.vector.tensor_copy(
                            attn_cols[:, c * ncc : (c + 1) * ncc], ps_pt
                        )

                        # ctx += p^T @ keys_chunk
                        for kc in range(O // 512):
                            ps_c = ps_ctx.tile([1, 512], dt, tag="ctx")
                            for jj in range(SC // P):
                                nc.tensor.matmul(
                                    ps_c,
                                    attn_cols[:, c * ncc + jj : c * ncc + jj + 1],
                                    knat[jj][:, kc * 512 : (kc + 1) * 512],
                                    start=(jj == 0), stop=(jj == SC // P - 1),
                                )
                            nc.vector.tensor_add(
                                ctxacc[0:1, kc * 512 : (kc + 1) * 512],
                                ctxacc[0:1, kc * 512 : (kc + 1) * 512],
                                ps_c,
                            )

                    # ---- batch epilogue ----
                    den = misc.tile([1, 1], dt, tag=f"den{b}")
                    nc.vector.reduce_sum(den, denp, axis=mybir.AxisListType.X)
                    inv = misc.tile([1, 1], dt, tag=f"inv{b}")
                    nc.vector.reciprocal(inv, den)
                    nc.vector.tensor_scalar_mul(ctxacc, ctxacc, inv)
                    nc.sync.dma_start(out=ctx_out[b : b + 1, :], in_=ctxacc)

                    ps_ib = ps_row.tile([P, 1], dt, tag="row")
                    nc.tensor.matmul(ps_ib, ones_row, inv, start=True, stop=True)
                    invb = misc.tile([P, 1], dt, tag=f"invb{b}")
                    nc.vector.tensor_copy(invb, ps_ib)
                    attn_sc = attnp.tile([P, s_total // P], dt, tag=f"as{b}")
                    nc.vector.tensor_scalar_mul(attn_sc, attn_cols, invb)
                    nc.sync.dma_start(out=attn_out[b], in_=attn_sc)

    nc.compile()
    return nc


def get_nc(bpc=BPC, s_total=S):
    key = (bpc, s_total)
    if key not in _CACHE:
        _CACHE[key] = _build(bpc, s_total)
    return _CACHE[key]


def _make_in_maps(inputs, bpc=BPC, s_total=S, ncores=NCORES):
    a = lambda x: np.ascontiguousarray(np.asarray(x, dtype=F32))
    query = a(inputs["query"]).reshape(-1, H)
    keys = a(inputs["keys"])
    wq, wqb = a(inputs["Wq_w"]), a(inputs["Wq_b"])
    wk, wkb = a(inputs["Wk_w"]), a(inputs["Wk_b"])
    vw = a(inputs["v_w"])
    in_maps = []
    for i in range(ncores):
        b0 = i * bpc
        in_maps.append(
            {
                "query": query[b0 : b0 + bpc],
                "keys": keys[b0 : b0 + bpc, :s_total],
                "wq": wq,
                "wqb": wqb,
                "wk": wk,
                "wkb": wkb,
                "vw": vw,
            }
        )
    return in_maps


def run(inputs, trace=False):
    from concourse.bass_utils import run_bass_kernel_spmd

    nc = get_nc()
    in_maps = _make_in_maps(inputs)
    res = run_bass_kernel_spmd(nc, in_maps, list(range(NCORES)), trace=trace)
    ctx = np.empty((B, 1, O), dtype=F32)
    attn = np.empty((B, S, 1), dtype=F32)
    for i in range(NCORES):
        r = res.results[i]
        for b in range(BPC):
            g = i * BPC + b
            ctx[g, 0, :] = r["ctx_out"][b]
            attn[g, :, 0] = r["attn_out"][b].T.reshape(-1)
    return (ctx, attn), res


def kernel(**inputs):
    (ctx, attn), _ = run(inputs, trace=False)
    return ctx, attn


# revision 8
# speedup vs baseline: 1.0897x; 1.0897x over previous
"""Bahdanau attention Trainium2 Bass kernel.

Problem: B=16, S=4096, H=1024, keys dim 2H=2048.
  q_proj = query @ Wq_w^T-ish (einsum bqh,oh->bqo) + Wq_b          [B,1,2048]
  k_proj = keys @ Wk_w^T (einsum bsk,ok->bso) + Wk_b               [B,S,2048]
  scores = tanh(q_proj + k_proj) @ v_w^T + v_b                     [B,S,1]
  attn   = softmax(scores, axis=S)                                 [B,S,1]
  ctx    = attn^T @ keys                                           [B,1,2048]
(v_b cancels in softmax - never used.)

Sharding: data-parallel over batch, 2 batches per core on 8 cores.

Per-core layout strategy (all fp32):
  - Wk^T [k,o] built once in SBUF via PE transposes (16 tiles [128,2048]).
  - Per 256-row s-chunk: keys rows DMA'd naturally, PE-transposed to
    keysT [k, s] tiles, matmul'd against WkT into k_projT [o, s-chunk]
    PSUM, ACT applies tanh with per-partition bias (q_proj + Wk_b),
    PE matvec with v gives scores [1, s-chunk], ACT exp (no max
    subtraction needed: |scores| <= ~36 so fp32 exp is safe) with
    accumulated denominator, PE K=1-transposes exp row into columns,
    and context accumulates via PE matmul against the still-resident
    natural keys tiles. Single pass over keys.
"""

import numpy as np

B, S, H, O = 16, 4096, 1024, 2048  # O = 2*H = keys feature dim
NCORES = 8
BPC = B // NCORES  # batches per core
P = 128
SC = 256  # s-chunk size
F32 = np.float32

_CACHE = {}
MM_DTYPE = "f32r"  # "f32" | "f32r" moving-operand dtype for PE matmuls


def _build(bpc, s_total, mm_dtype=None):
    import concourse.bass as bass
    import concourse.tile as tile
    from concourse import bacc, mybir
    from concourse.masks import make_identity

    dt = mybir.dt.float32
    if mm_dtype is None:
        mm_dtype = MM_DTYPE
    mdt = mybir.dt.float32r if mm_dtype == "f32r" else mybir.dt.float32
    use_r = mm_dtype == "f32r"
    # producers write f32r-typed tiles so walrus sees rounded inputs; the
    # dtype cast rides existing DVE/ACT copies and the SWDGE keys loads.
    kdma = (lambda **kw: nc.gpsimd.dma_start(**kw)) if use_r else (
        lambda **kw: nc.sync.dma_start(**kw))
    AF = mybir.ActivationFunctionType
    n_chunks = s_total // SC
    KT = O // P   # 16 k-tiles (keys feature dim)
    OT = O // P   # 16 o-tiles (projection dim)
    HT = H // P   # 8 h-tiles (query dim)

    nc = bacc.Bacc("TRN2", target_bir_lowering=False, debug=False)

    q_in = nc.dram_tensor("query", [bpc, H], dt, kind="ExternalInput")
    k_in = nc.dram_tensor("keys", [bpc, s_total, O], dt, kind="ExternalInput")
    wq_in = nc.dram_tensor("wq", [O, H], dt, kind="ExternalInput")
    wqb_in = nc.dram_tensor("wqb", [O], dt, kind="ExternalInput")
    wk_in = nc.dram_tensor("wk", [O, O], dt, kind="ExternalInput")
    wkb_in = nc.dram_tensor("wkb", [O], dt, kind="ExternalInput")
    v_in = nc.dram_tensor("vw", [1, O], dt, kind="ExternalInput")
    ctx_out = nc.dram_tensor("ctx_out", [bpc, O], dt, kind="ExternalOutput")
    attn_out = nc.dram_tensor(
        "attn_out", [bpc, P, s_total // P], dt, kind="ExternalOutput"
    )

    with tile.TileContext(nc) as tc:
        with (
            tc.tile_pool(name="wkt", bufs=1) as wkt_pool,
            tc.tile_pool(name="knat", bufs=4) as knat_pool,
            tc.tile_pool(name="misc", bufs=1) as misc,
            tc.tile_pool(name="ps_tr", bufs=2, space="PSUM") as ps_tr,
            tc.tile_pool(name="ps_mm", bufs=2, space="PSUM") as ps_mm,
            tc.tile_pool(name="ps_row", bufs=2, space="PSUM") as ps_row,
            tc.tile_pool(name="ps_ctx", bufs=2, space="PSUM") as ps_ctx,
        ):
            ident_f = misc.tile([P, P], dt, tag="ident_f")
            make_identity(nc, ident_f)
            ident = misc.tile([P, P], mdt, tag="ident")
            nc.vector.tensor_copy(ident, ident_f)
            ones1 = misc.tile([1, 1], dt, tag="ones1")
            nc.gpsimd.memset(ones1, 1.0)
            ones_row = misc.tile([1, P], dt, tag="ones_row")
            nc.gpsimd.memset(ones_row, 1.0)

            # ---- init phase: weights transposes + per-batch bias columns ----
            vcol = misc.tile([P, OT], mdt, tag="vcol")
            qcols = [misc.tile([P, HT], dt, tag=f"qcols{b}", name=f"qcols{b}") for b in range(bpc)]
            biascol = [misc.tile([P, OT], dt, tag=f"biascol{b}", name=f"biascol{b}") for b in range(bpc)]

            with tc.tile_pool(name="rows", bufs=2) as rows_pool, tc.tile_pool(
                name="wblk", bufs=3
            ) as wblk_pool:
                # v as columns: v[m*128+p] -> vcol[p, m]
                vrow = rows_pool.tile([1, O], dt, tag="row")
                nc.sync.dma_start(out=vrow, in_=v_in[0:1, :])
                ps_v = ps_row.tile([P, OT], dt, tag="row")
                for m in range(OT):
                    nc.tensor.matmul(
                        ps_v[:, m : m + 1], vrow[0:1, m * P : (m + 1) * P], ones1,
                        start=True, stop=True,
                    )
                nc.vector.tensor_copy(vcol, ps_v)

                # query as columns per batch
                for b in range(bpc):
                    qrow = rows_pool.tile([1, H], dt, tag="row")
                    nc.sync.dma_start(out=qrow, in_=q_in[b : b + 1, :])
                    ps_q = ps_row.tile([P, HT], dt, tag="row")
                    for i in range(HT):
                        nc.tensor.matmul(
                            ps_q[:, i : i + 1], qrow[0:1, i * P : (i + 1) * P], ones1,
                            start=True, stop=True,
                        )
                    nc.vector.tensor_copy(qcols[b], ps_q)

                wqb_row = rows_pool.tile([1, O], dt, tag="row")
                nc.sync.dma_start(out=wqb_row, in_=wqb_in[None, :])
                wkb_row = rows_pool.tile([1, O], dt, tag="row")
                nc.sync.dma_start(out=wkb_row, in_=wkb_in[None, :])

                # bias columns: biascol[b][:, j] = (Wq @ query_b + Wq_b + Wk_b)[o-tile j]
                ps_bias = [ps_mm.tile([P, OT], dt, tag="mm", name=f"ps_bias{b}") for b in range(bpc)]
                for j in range(OT):
                    wq_nat = knat_pool.tile([P, H], dt, tag="knat")
                    nc.sync.dma_start(out=wq_nat, in_=wq_in[j * P : (j + 1) * P, :])
                    for i in range(HT):
                        ps_w = ps_tr.tile([P, P], dt, tag="tr")
                        nc.tensor.transpose(
                            ps_w, wq_nat[:, i * P : (i + 1) * P], ident_f
                        )
                        wqt_blk = wblk_pool.tile([P, P], dt, tag="wblk")
                        nc.vector.tensor_copy(wqt_blk, ps_w)
                        for b in range(bpc):
                            nc.tensor.matmul(
                                ps_bias[b][:, j : j + 1],
                                wqt_blk,
                                qcols[b][:, i : i + 1],
                                start=(i == 0), stop=False,
                            )
                    for b in range(bpc):
                        nc.tensor.matmul(
                            ps_bias[b][:, j : j + 1],
                            wqb_row[0:1, j * P : (j + 1) * P],
                            ones1,
                            start=False, stop=False,
                        )
                        nc.tensor.matmul(
                            ps_bias[b][:, j : j + 1],
                            wkb_row[0:1, j * P : (j + 1) * P],
                            ones1,
                            start=False, stop=True,
                        )
                for b in range(bpc):
                    nc.vector.tensor_copy(biascol[b], ps_bias[b])

                # WkT resident tiles: wkt[kk][p, :] holds Wk[:, kk*128+p]
                wkt = [wkt_pool.tile([P, O], mdt, tag=f"wkt{kk}", name=f"wkt{kk}") for kk in range(KT)]
                for j in range(OT):
                    wk_nat = knat_pool.tile([P, O], mdt, tag="knat")
                    kdma(out=wk_nat, in_=wk_in[j * P : (j + 1) * P, :])
                    for kk in range(KT):
                        ps_w = ps_tr.tile([P, P], mdt, tag="tr")
                        nc.tensor.transpose(
                            ps_w, wk_nat[:, kk * P : (kk + 1) * P], ident
                        )
                        nc.vector.tensor_copy(
                            wkt[kk][:, j * P : (j + 1) * P], ps_w
                        )

            # ---- main loop ----
            with (
                tc.tile_pool(name="ktr", bufs=KT) as ktr_pool,
                tc.tile_pool(name="tt", bufs=OT) as tt_pool,
                tc.tile_pool(name="prow", bufs=2) as prow_pool,
                tc.tile_pool(name="ctxp", bufs=1) as ctxp,
                tc.tile_pool(name="attnp", bufs=1) as attnp,
            ):
                for b in range(bpc):
                    ctxacc = ctxp.tile([1, O], dt, tag="ctxacc")
                    nc.gpsimd.memset(ctxacc, 0.0)
                    attn_cols = attnp.tile([P, s_total // P], dt, tag=f"ac{b}")
                    denp = misc.tile([1, n_chunks], dt, tag=f"denp{b}")

                    for c in range(n_chunks):
                        knat = []
                        for jj in range(SC // P):
                            kn = knat_pool.tile([P, O], mdt, tag="knat", name="kn")
                            s0 = c * SC + jj * P
                            kdma(out=kn, in_=k_in[b, s0 : s0 + P, :])
                            knat.append(kn)

                        # transpose keys chunk -> keysT [k, s-chunk]
                        ktr = []
                        for kk in range(KT):
                            ps_t = ps_tr.tile([P, SC], mdt, tag="tr")
                            for jj in range(SC // P):
                                nc.tensor.transpose(
                                    ps_t[:, jj * P : (jj + 1) * P],
                                    knat[jj][:, kk * P : (kk + 1) * P],
                                    ident,
                                )
                            kt = ktr_pool.tile([P, SC], mdt, tag="ktr")
                            nc.vector.tensor_copy(kt, ps_t)
                            ktr.append(kt)

                        # k_projT [o-tile, s-chunk] + tanh
                        tts = []
                        for m in range(OT):
                            ps_k = ps_mm.tile([P, SC], dt, tag="mm")
                            for kk in range(KT):
                                nc.tensor.matmul(
                                    ps_k,
                                    wkt[kk][:, m * P : (m + 1) * P],
                                    ktr[kk],
                                    start=(kk == 0), stop=(kk == KT - 1),
                                )
                            t_m = tt_pool.tile([P, SC], mdt, tag="tt")
                            nc.scalar.activation(
                                t_m, ps_k, AF.Tanh, bias=biascol[b][:, m : m + 1]
                            )
                            tts.append(t_m)

                        # scores [1, s-chunk]
                        ps_sc = ps_row.tile([1, SC], dt, tag="row")
                        for m in range(OT):
                            nc.tensor.matmul(
                                ps_sc, vcol[:, m : m + 1], tts[m],
                                start=(m == 0), stop=(m == OT - 1),
                            )

                        # p = exp(scores); denominator partial
                        prow = prow_pool.tile([1, SC], dt, tag="prow")
                        nc.scalar.activation(
                            prow, ps_sc, AF.Exp, accum_out=denp[:, c : c + 1]
                        )

                        # p row -> columns, into attn_cols
                        ps_pt = ps_row.tile([P, SC // P], dt, tag="row")
                        for jj in range(SC // P):
                            nc.tensor.matmul(
                                ps_pt[:, jj : jj + 1],
                                prow[0:1, jj * P : (jj + 1) * P],
                                ones1,
                                start=True, stop=True,
                            )
                        ncc = SC // P
                        nc.vector.tensor_copy(
                            attn_cols[:, c * ncc : (c + 1) * ncc], ps_pt
                        )
                        pcol_r = prow_pool.tile([P, SC // P], mdt, tag="pcol")
                        nc.vector.tensor_copy(pcol_r, ps_pt)

                        # ctx += p^T @ keys_chunk
                        for kc in range(O // 512):
                            ps_c = ps_ctx.tile([1, 512], dt, tag="ctx")
                            for jj in range(SC // P):
                                nc.tensor.matmul(
                                    ps_c,
                                    pcol_r[:, jj : jj + 1],
                                    knat[jj][:, kc * 512 : (kc + 1) * 512],
                                    start=(jj == 0), stop=(jj == SC // P - 1),
                                )
                            nc.vector.tensor_add(
                                ctxacc[0:1, kc * 512 : (kc + 1) * 512],
                                ctxacc[0:1, kc * 512 : (kc + 1) * 512],
                                ps_c,
                            )

                    # ---- batch epilogue ----
                    den = misc.tile([1, 1], dt, tag=f"den{b}")
                    nc.vector.reduce_sum(den, denp, axis=mybir.AxisListType.X)
                    inv = misc.tile([1, 1], dt, tag=f"inv{b}")
                    nc.vector.reciprocal(inv, den)
                    nc.vector.tensor_scalar_mul(ctxacc, ctxacc, inv)
                    nc.sync.dma_start(out=ctx_out[b : b + 1, :], in_=ctxacc)

                    ps_ib = ps_row.tile([P, 1], dt, tag="row")
                    nc.tensor.matmul(ps_ib, ones_row, inv, start=True, stop=True)
                    invb = misc.tile([P, 1], dt, tag=f"invb{b}")
                    nc.vector.tensor_copy(invb, ps_ib)
                    attn_sc = attnp.tile([P, s_total // P], dt, tag=f"as{b}")
                    nc.vector.tensor_scalar_mul(attn_sc, attn_cols, invb)
                    nc.sync.dma_start(out=attn_out[b], in_=attn_sc)

    nc.compile()
    return nc


def get_nc(bpc=BPC, s_total=S, mm_dtype=None):
    if mm_dtype is None:
        mm_dtype = MM_DTYPE
    key = (bpc, s_total, mm_dtype)
    if key not in _CACHE:
        _CACHE[key] = _build(bpc, s_total, mm_dtype)
    return _CACHE[key]


def _make_in_maps(inputs, bpc=BPC, s_total=S, ncores=NCORES):
    a = lambda x: np.ascontiguousarray(np.asarray(x, dtype=F32))
    query = a(inputs["query"]).reshape(-1, H)
    keys = a(inputs["keys"])
    wq, wqb = a(inputs["Wq_w"]), a(inputs["Wq_b"])
    wk, wkb = a(inputs["Wk_w"]), a(inputs["Wk_b"])
    vw = a(inputs["v_w"])
    in_maps = []
    for i in range(ncores):
        b0 = i * bpc
        in_maps.append(
            {
                "query": query[b0 : b0 + bpc],
                "keys": keys[b0 : b0 + bpc, :s_total],
                "wq": wq,
                "wqb": wqb,
                "wk": wk,
                "wkb": wkb,
                "vw": vw,
            }
        )
    return in_maps


def run(inputs, trace=False):
    from concourse.bass_utils import run_bass_kernel_spmd

    nc = get_nc()
    in_maps = _make_in_maps(inputs)
    res = run_bass_kernel_spmd(nc, in_maps, list(range(NCORES)), trace=trace)
    ctx = np.empty((B, 1, O), dtype=F32)
    attn = np.empty((B, S, 1), dtype=F32)
    for i in range(NCORES):
        r = res.results[i]
        for b in range(BPC):
            g = i * BPC + b
            ctx[g, 0, :] = r["ctx_out"][b]
            attn[g, :, 0] = r["attn_out"][b].T.reshape(-1)
    return (ctx, attn), res


def kernel(**inputs):
    (ctx, attn), _ = run(inputs, trace=False)
    return ctx, attn


# revision 10
# speedup vs baseline: 1.8788x; 1.7240x over previous
"""Bahdanau attention Trainium2 Bass kernel.

Problem: B=16, S=4096, H=1024, keys dim 2H=2048.
  q_proj = query @ Wq_w^T-ish (einsum bqh,oh->bqo) + Wq_b          [B,1,2048]
  k_proj = keys @ Wk_w^T (einsum bsk,ok->bso) + Wk_b               [B,S,2048]
  scores = tanh(q_proj + k_proj) @ v_w^T + v_b                     [B,S,1]
  attn   = softmax(scores, axis=S)                                 [B,S,1]
  ctx    = attn^T @ keys                                           [B,1,2048]
(v_b cancels in softmax - never used.)

Sharding: data-parallel over batch, 2 batches per core on 8 cores.

Per-core layout strategy (all fp32):
  - Wk^T [k,o] built once in SBUF via PE transposes (16 tiles [128,2048]).
  - Per 256-row s-chunk: keys rows DMA'd naturally, PE-transposed to
    keysT [k, s] tiles, matmul'd against WkT into k_projT [o, s-chunk]
    PSUM, ACT applies tanh with per-partition bias (q_proj + Wk_b),
    PE matvec with v gives scores [1, s-chunk], ACT exp (no max
    subtraction needed: |scores| <= ~36 so fp32 exp is safe) with
    accumulated denominator, PE K=1-transposes exp row into columns,
    and context accumulates via PE matmul against the still-resident
    natural keys tiles. Single pass over keys.
"""

import numpy as np

B, S, H, O = 16, 4096, 1024, 2048  # O = 2*H = keys feature dim
NCORES = 8
BPC = B // NCORES  # batches per core
P = 128
SC = 256  # s-chunk size
F32 = np.float32

_CACHE = {}
MM_DTYPE = "f32r"  # "f32" | "f32r" moving-operand dtype for PE matmuls


def _build(bpc, s_total, mm_dtype=None, repeat=1):
    import concourse.bass as bass
    import concourse.tile as tile
    from concourse import bacc, mybir
    from concourse.masks import make_identity

    dt = mybir.dt.float32
    if mm_dtype is None:
        mm_dtype = MM_DTYPE
    mdt = mybir.dt.float32r if mm_dtype == "f32r" else mybir.dt.float32
    use_r = mm_dtype == "f32r"
    # producers write f32r-typed tiles so walrus sees rounded inputs; the
    # dtype cast rides existing DVE/ACT copies and the SWDGE keys loads.
    kdma = (lambda **kw: nc.gpsimd.dma_start(**kw)) if use_r else (
        lambda **kw: nc.sync.dma_start(**kw))
    AF = mybir.ActivationFunctionType
    n_chunks = s_total // SC
    KT = O // P   # 16 k-tiles (keys feature dim)
    OT = O // P   # 16 o-tiles (projection dim)
    HT = H // P   # 8 h-tiles (query dim)

    nc = bacc.Bacc("TRN2", target_bir_lowering=False, debug=False)

    q_in = nc.dram_tensor("query", [bpc, H], dt, kind="ExternalInput")
    k_in = nc.dram_tensor("keys", [bpc, s_total, O], dt, kind="ExternalInput")
    wq_in = nc.dram_tensor("wq", [O, H], dt, kind="ExternalInput")
    wqb_in = nc.dram_tensor("wqb", [O], dt, kind="ExternalInput")
    wk_in = nc.dram_tensor("wk", [O, O], dt, kind="ExternalInput")
    wkb_in = nc.dram_tensor("wkb", [O], dt, kind="ExternalInput")
    v_in = nc.dram_tensor("vw", [1, O], dt, kind="ExternalInput")
    ctx_out = nc.dram_tensor("ctx_out", [bpc, O], dt, kind="ExternalOutput")
    attn_out = nc.dram_tensor(
        "attn_out", [bpc, P, s_total // P], dt, kind="ExternalOutput"
    )

    with tile.TileContext(nc) as tc:
        with (
            tc.tile_pool(name="wkt", bufs=1) as wkt_pool,
            tc.tile_pool(name="knat", bufs=4) as knat_pool,
            tc.tile_pool(name="misc", bufs=1) as misc,
            tc.tile_pool(name="ps_tr", bufs=2, space="PSUM") as ps_tr,
            tc.tile_pool(name="ps_mm", bufs=2, space="PSUM") as ps_mm,
            tc.tile_pool(name="ps_row", bufs=2, space="PSUM") as ps_row,
            tc.tile_pool(name="ps_ctx", bufs=2, space="PSUM") as ps_ctx,
        ):
            ident_f = misc.tile([P, P], dt, tag="ident_f")
            make_identity(nc, ident_f)
            ident = misc.tile([P, P], mdt, tag="ident")
            nc.vector.tensor_copy(ident, ident_f)
            ones1 = misc.tile([1, 1], dt, tag="ones1")
            nc.gpsimd.memset(ones1, 1.0)
            ones_row = misc.tile([1, P], dt, tag="ones_row")
            nc.gpsimd.memset(ones_row, 1.0)

            # ---- init phase: weights transposes + per-batch bias columns ----
            vcol = misc.tile([P, OT], mdt, tag="vcol")
            qcols = [misc.tile([P, HT], dt, tag=f"qcols{b}", name=f"qcols{b}") for b in range(bpc)]
            biascol = [misc.tile([P, OT], dt, tag=f"biascol{b}", name=f"biascol{b}") for b in range(bpc)]

            with tc.tile_pool(name="rows", bufs=2) as rows_pool, tc.tile_pool(
                name="wblk", bufs=3
            ) as wblk_pool:
                # v as columns: v[m*128+p] -> vcol[p, m]
                vrow = rows_pool.tile([1, O], dt, tag="row")
                nc.sync.dma_start(out=vrow, in_=v_in[0:1, :])
                ps_v = ps_row.tile([P, OT], dt, tag="row")
                for m in range(OT):
                    nc.tensor.matmul(
                        ps_v[:, m : m + 1], vrow[0:1, m * P : (m + 1) * P], ones1,
                        start=True, stop=True,
                    )
                nc.vector.tensor_copy(vcol, ps_v)

                # query as columns per batch
                for b in range(bpc):
                    qrow = rows_pool.tile([1, H], dt, tag="row")
                    nc.sync.dma_start(out=qrow, in_=q_in[b : b + 1, :])
                    ps_q = ps_row.tile([P, HT], dt, tag="row")
                    for i in range(HT):
                        nc.tensor.matmul(
                            ps_q[:, i : i + 1], qrow[0:1, i * P : (i + 1) * P], ones1,
                            start=True, stop=True,
                        )
                    nc.vector.tensor_copy(qcols[b], ps_q)

                wqb_row = rows_pool.tile([1, O], dt, tag="row")
                nc.sync.dma_start(out=wqb_row, in_=wqb_in[None, :])
                wkb_row = rows_pool.tile([1, O], dt, tag="row")
                nc.sync.dma_start(out=wkb_row, in_=wkb_in[None, :])

                # bias columns: biascol[b][:, j] = (Wq @ query_b + Wq_b + Wk_b)[o-tile j]
                ps_bias = [ps_mm.tile([P, OT], dt, tag="mm", name=f"ps_bias{b}") for b in range(bpc)]
                for j in range(OT):
                    wq_nat = knat_pool.tile([P, H], dt, tag="knat")
                    nc.sync.dma_start(out=wq_nat, in_=wq_in[j * P : (j + 1) * P, :])
                    for i in range(HT):
                        ps_w = ps_tr.tile([P, P], dt, tag="tr")
                        nc.tensor.transpose(
                            ps_w, wq_nat[:, i * P : (i + 1) * P], ident_f
                        )
                        wqt_blk = wblk_pool.tile([P, P], dt, tag="wblk")
                        nc.vector.tensor_copy(wqt_blk, ps_w)
                        for b in range(bpc):
                            nc.tensor.matmul(
                                ps_bias[b][:, j : j + 1],
                                wqt_blk,
                                qcols[b][:, i : i + 1],
                                start=(i == 0), stop=False,
                            )
                    for b in range(bpc):
                        nc.tensor.matmul(
                            ps_bias[b][:, j : j + 1],
                            wqb_row[0:1, j * P : (j + 1) * P],
                            ones1,
                            start=False, stop=False,
                        )
                        nc.tensor.matmul(
                            ps_bias[b][:, j : j + 1],
                            wkb_row[0:1, j * P : (j + 1) * P],
                            ones1,
                            start=False, stop=True,
                        )
                for b in range(bpc):
                    nc.vector.tensor_copy(biascol[b], ps_bias[b])

                # WkT resident tiles: wkt[kk][p, :] holds Wk[:, kk*128+p]
                wkt = [wkt_pool.tile([P, O], mdt, tag=f"wkt{kk}", name=f"wkt{kk}") for kk in range(KT)]
                for j in range(OT):
                    wk_nat = knat_pool.tile([P, O], mdt, tag="knat")
                    kdma(out=wk_nat, in_=wk_in[j * P : (j + 1) * P, :])
                    for kk in range(KT):
                        ps_w = ps_tr.tile([P, P], mdt, tag="tr")
                        nc.tensor.transpose(
                            ps_w, wk_nat[:, kk * P : (kk + 1) * P], ident
                        )
                        nc.vector.tensor_copy(
                            wkt[kk][:, j * P : (j + 1) * P], ps_w
                        )

            # ---- main loop ----
            with (
                tc.tile_pool(name="ktr", bufs=KT) as ktr_pool,
                tc.tile_pool(name="tt", bufs=OT) as tt_pool,
                tc.tile_pool(name="prow", bufs=2) as prow_pool,
                tc.tile_pool(name="ctxp", bufs=1) as ctxp,
                tc.tile_pool(name="attnp", bufs=1) as attnp,
            ):
              for _rep in range(repeat):
                for b in range(bpc):
                    ctxacc = ctxp.tile([1, O], dt, tag="ctxacc")
                    nc.gpsimd.memset(ctxacc, 0.0)
                    attn_cols = attnp.tile([P, s_total // P], dt, tag=f"ac{b}")
                    denp = misc.tile([1, n_chunks], dt, tag=f"denp{b}")

                    def chunk_tail(c, knat, prow, ncc):
                        # p row -> columns, into attn_cols (f32) and pcol_r
                        # (f32r) for the ctx matmul; then ctx += p^T @ keys.
                        # Emitted one chunk late so the ACT exp hides under
                        # the next chunk's PE transposes.
                        ps_pt = ps_row.tile([P, SC // P], dt, tag="row")
                        for jj in range(SC // P):
                            nc.tensor.matmul(
                                ps_pt[:, jj : jj + 1],
                                prow[0:1, jj * P : (jj + 1) * P],
                                ones1,
                                start=True, stop=True,
                            )
                        nc.vector.tensor_copy(
                            attn_cols[:, c * ncc : (c + 1) * ncc], ps_pt
                        )
                        pcol_r = prow_pool.tile([P, SC // P], mdt, tag="pcol")
                        nc.vector.tensor_copy(pcol_r, ps_pt)
                        for kc in range(O // 512):
                            ps_c = ps_ctx.tile([1, 512], dt, tag="ctx")
                            for jj in range(SC // P):
                                nc.tensor.matmul(
                                    ps_c,
                                    pcol_r[:, jj : jj + 1],
                                    knat[jj][:, kc * 512 : (kc + 1) * 512],
                                    start=(jj == 0), stop=(jj == SC // P - 1),
                                )
                            nc.vector.tensor_add(
                                ctxacc[0:1, kc * 512 : (kc + 1) * 512],
                                ctxacc[0:1, kc * 512 : (kc + 1) * 512],
                                ps_c,
                            )

                    pending = None
                    for c in range(n_chunks):
                        knat = []
                        for jj in range(SC // P):
                            kn = knat_pool.tile([P, O], mdt, tag="knat", name="kn")
                            s0 = c * SC + jj * P
                            kdma(out=kn, in_=k_in[b, s0 : s0 + P, :])
                            knat.append(kn)

                        # transpose keys chunk -> keysT [k, s-chunk]
                        ktr = []
                        for kk in range(KT):
                            ps_t = ps_tr.tile([P, SC], mdt, tag="tr")
                            for jj in range(SC // P):
                                nc.tensor.transpose(
                                    ps_t[:, jj * P : (jj + 1) * P],
                                    knat[jj][:, kk * P : (kk + 1) * P],
                                    ident,
                                )
                            kt = ktr_pool.tile([P, SC], mdt, tag="ktr")
                            nc.vector.tensor_copy(kt, ps_t)
                            ktr.append(kt)

                        if pending is not None:
                            chunk_tail(*pending)
                            pending = None

                        # k_projT [o-tile, s-chunk] + tanh
                        tts = []
                        for m in range(OT):
                            ps_k = ps_mm.tile([P, SC], dt, tag="mm")
                            for kk in range(KT):
                                nc.tensor.matmul(
                                    ps_k,
                                    wkt[kk][:, m * P : (m + 1) * P],
                                    ktr[kk],
                                    start=(kk == 0), stop=(kk == KT - 1),
                                )
                            t_m = tt_pool.tile([P, SC], mdt, tag="tt")
                            nc.scalar.activation(
                                t_m, ps_k, AF.Tanh, bias=biascol[b][:, m : m + 1]
                            )
                            tts.append(t_m)

                        # scores [1, s-chunk]
                        ps_sc = ps_row.tile([1, SC], dt, tag="row")
                        for m in range(OT):
                            nc.tensor.matmul(
                                ps_sc, vcol[:, m : m + 1], tts[m],
                                start=(m == 0), stop=(m == OT - 1),
                            )

                        # p = exp(scores); denominator partial
                        prow = prow_pool.tile([1, SC], dt, tag="prow")
                        nc.scalar.activation(
                            prow, ps_sc, AF.Exp, accum_out=denp[:, c : c + 1]
                        )
                        pending = (c, knat, prow, SC // P)

                    if pending is not None:
                        chunk_tail(*pending)
                        pending = None

                    # ---- batch epilogue ----
                    den = misc.tile([1, 1], dt, tag=f"den{b}")
                    nc.vector.reduce_sum(den, denp, axis=mybir.AxisListType.X)
                    inv = misc.tile([1, 1], dt, tag=f"inv{b}")
                    nc.vector.reciprocal(inv, den)
                    nc.vector.tensor_scalar_mul(ctxacc, ctxacc, inv)
                    nc.sync.dma_start(out=ctx_out[b : b + 1, :], in_=ctxacc)

                    ps_ib = ps_row.tile([P, 1], dt, tag="row")
                    nc.tensor.matmul(ps_ib, ones_row, inv, start=True, stop=True)
                    invb = misc.tile([P, 1], dt, tag=f"invb{b}")
                    nc.vector.tensor_copy(invb, ps_ib)
                    attn_sc = attnp.tile([P, s_total // P], dt, tag=f"as{b}")
                    nc.vector.tensor_scalar_mul(attn_sc, attn_cols, invb)
                    nc.sync.dma_start(out=attn_out[b], in_=attn_sc)

    nc.compile()
    return nc


def get_nc(bpc=BPC, s_total=S, mm_dtype=None, repeat=1):
    if mm_dtype is None:
        mm_dtype = MM_DTYPE
    key = (bpc, s_total, mm_dtype, repeat)
    if key not in _CACHE:
        _CACHE[key] = _build(bpc, s_total, mm_dtype, repeat)
    return _CACHE[key]


def _make_in_maps(inputs, bpc=BPC, s_total=S, ncores=NCORES):
    a = lambda x: np.ascontiguousarray(np.asarray(x, dtype=F32))
    query = a(inputs["query"]).reshape(-1, H)
    keys = a(inputs["keys"])
    wq, wqb = a(inputs["Wq_w"]), a(inputs["Wq_b"])
    wk, wkb = a(inputs["Wk_w"]), a(inputs["Wk_b"])
    vw = a(inputs["v_w"])
    in_maps = []
    for i in range(ncores):
        b0 = i * bpc
        in_maps.append(
            {
                "query": query[b0 : b0 + bpc],
                "keys": keys[b0 : b0 + bpc, :s_total],
                "wq": wq,
                "wqb": wqb,
                "wk": wk,
                "wkb": wkb,
                "vw": vw,
            }
        )
    return in_maps


def run(inputs, trace=False):
    from concourse.bass_utils import run_bass_kernel_spmd

    nc = get_nc()
    in_maps = _make_in_maps(inputs)
    res = run_bass_kernel_spmd(nc, in_maps, list(range(NCORES)), trace=trace)
    ctx = np.empty((B, 1, O), dtype=F32)
    attn = np.empty((B, S, 1), dtype=F32)
    for i in range(NCORES):
        r = res.results[i]
        for b in range(BPC):
            g = i * BPC + b
            ctx[g, 0, :] = r["ctx_out"][b]
            attn[g, :, 0] = r["attn_out"][b].T.reshape(-1)
    return (ctx, attn), res


def kernel(**inputs):
    (ctx, attn), _ = run(inputs, trace=False)
    return ctx, attn


# revision 14
# speedup vs baseline: 8.3245x; 4.4309x over previous
"""Bahdanau attention Trainium2 Bass kernel.

Problem: B=16, S=4096, H=1024, keys dim 2H=2048.
  q_proj = query @ Wq_w^T-ish (einsum bqh,oh->bqo) + Wq_b          [B,1,2048]
  k_proj = keys @ Wk_w^T (einsum bsk,ok->bso) + Wk_b               [B,S,2048]
  scores = tanh(q_proj + k_proj) @ v_w^T + v_b                     [B,S,1]
  attn   = softmax(scores, axis=S)                                 [B,S,1]
  ctx    = attn^T @ keys                                           [B,1,2048]
(v_b cancels in softmax - never used.)

Sharding: data-parallel over batch, 2 batches per core on 8 cores.

Per-core strategy (fp32 data, float32r PE matmuls ~ 4x faster, err ~2e-4):
  - Wk^T [k,o] resident in SBUF (built once via PE transposes).
  - Per 256-row s-chunk: k_projT[o,s] = WkT-tiles x keysT-tiles into PSUM,
    tanh fused on ACT with per-partition bias (q_proj+Wk_b), PE matvec with
    v gives scores [1,sc], exp on ACT (no max subtraction needed; scores
    are O(1)) with fused denominator accumulation, exp-row transposed to
    columns via K=1 matmuls, context accumulated in PSUM over the whole
    batch against natural-layout keys rows. Keys make one natural pass
    (for ctx) plus, in "dve" mode, one block-permuted pass feeding DVE
    32x32 transposes (keeps the PE free of transpose work); in "pe" mode
    keysT comes from PE transpose-mode instead.
  - Softmax tail of chunk c is emitted during chunk c+1 so ACT work hides
    under PE work.
"""

import numpy as np

B, S, H, O = 16, 4096, 1024, 2048  # O = 2*H = keys feature dim
NCORES = 8
BPC = B // NCORES  # batches per core
P = 128
SC = 256  # s-chunk size
F32 = np.float32

_CACHE = {}
MM_DTYPE = "f32r"  # "f32" | "f32r" dtype for PE matmul operands
TRANS = "pei"  # "pe" | "pei" | "dve" keys-transpose path


def _build(bpc, s_total, mm_dtype=None, repeat=1, trans=None):
    import concourse.bass as bass
    import concourse.tile as tile
    from concourse import bacc, mybir
    from concourse.masks import make_identity

    dt = mybir.dt.float32
    if mm_dtype is None:
        mm_dtype = MM_DTYPE
    if trans is None:
        trans = TRANS
    use_r = mm_dtype == "f32r"
    mdt = mybir.dt.float32r if use_r else mybir.dt.float32
    # producers write f32r-typed tiles so walrus sees rounded matmul inputs;
    # the cast rides existing DVE/ACT copies and the SWDGE keys loads.
    AF = mybir.ActivationFunctionType
    n_chunks = s_total // SC
    KT = O // P   # 16 k-tiles
    OT = O // P   # 16 o-tiles
    HT = H // P   # 8 h-tiles
    NJ = SC // P  # s-subtiles per chunk

    nc = bacc.Bacc("TRN2", target_bir_lowering=False, debug=False)
    kdma = (lambda **kw: nc.gpsimd.dma_start(**kw)) if use_r else (
        lambda **kw: nc.sync.dma_start(**kw))

    q_in = nc.dram_tensor("query", [bpc, H], dt, kind="ExternalInput")
    k_in = nc.dram_tensor("keys", [bpc, s_total, O], dt, kind="ExternalInput")
    wq_in = nc.dram_tensor("wq", [O, H], dt, kind="ExternalInput")
    wqb_in = nc.dram_tensor("wqb", [O], dt, kind="ExternalInput")
    wk_in = nc.dram_tensor("wk", [O, O], dt, kind="ExternalInput")
    wkb_in = nc.dram_tensor("wkb", [O], dt, kind="ExternalInput")
    v_in = nc.dram_tensor("vw", [1, O], dt, kind="ExternalInput")
    ctx_out = nc.dram_tensor("ctx_out", [bpc, O], dt, kind="ExternalOutput")
    attn_out = nc.dram_tensor(
        "attn_out", [bpc, P, s_total // P], dt, kind="ExternalOutput"
    )
    # block-permuted view for the DVE-transpose path:
    # kvs[b][c, q, I][bb, kk4, J, aa] = keys[b, c*SC+J*32+bb, q*512+kk4*128+I*32+aa]
    kvs = [
        k_in[bb_].rearrange(
            "(c J bb) (q kk4 I aa) -> c q I bb kk4 J aa",
            J=SC // 32, bb=32, q=4, kk4=4, I=4, aa=32,
        )
        for bb_ in range(bpc)
    ]

    with tile.TileContext(nc) as tc:
        with (
            tc.tile_pool(name="wkt", bufs=1) as wkt_pool,
            tc.tile_pool(name="knat", bufs=4) as knat_pool,
            tc.tile_pool(name="misc", bufs=1) as misc,
            tc.tile_pool(name="ps_mm", bufs=2, space="PSUM") as ps_mm,
            tc.tile_pool(name="ps_row", bufs=2, space="PSUM") as ps_row,
        ):
            ones1 = misc.tile([1, 1], dt, tag="ones1")
            nc.gpsimd.memset(ones1, 1.0)
            ones_row = misc.tile([1, P], dt, tag="ones_row")
            nc.gpsimd.memset(ones_row, 1.0)
            vcol = misc.tile([P, OT], mdt, tag="vcol")
            qcols = [misc.tile([P, HT], dt, tag=f"qcols{b}", name=f"qcols{b}")
                     for b in range(bpc)]
            biascol = [misc.tile([P, OT], dt, tag=f"biascol{b}", name=f"biascol{b}")
                       for b in range(bpc)]
            ident = misc.tile([P, P], mdt, tag="ident")

            # ---- init: weight transposes + per-batch bias columns ----
            with (
                tc.tile_pool(name="rows", bufs=2) as rows_pool,
                tc.tile_pool(name="wblk", bufs=3) as wblk_pool,
                tc.tile_pool(name="ps_tr", bufs=2, space="PSUM") as ps_tr,
            ):
                ident_f = rows_pool.tile([P, P], dt, tag="identf")
                make_identity(nc, ident_f)
                nc.vector.tensor_copy(ident, ident_f)

                vrow = rows_pool.tile([1, O], dt, tag="row")
                nc.sync.dma_start(out=vrow, in_=v_in[0:1, :])
                ps_v = ps_row.tile([P, OT], dt, tag="row")
                for m in range(OT):
                    nc.tensor.matmul(
                        ps_v[:, m : m + 1], vrow[0:1, m * P : (m + 1) * P], ones1,
                        start=True, stop=True,
                    )
                nc.vector.tensor_copy(vcol, ps_v)

                for b in range(bpc):
                    qrow = rows_pool.tile([1, H], dt, tag="row")
                    nc.sync.dma_start(out=qrow, in_=q_in[b : b + 1, :])
                    ps_q = ps_row.tile([P, HT], dt, tag="row")
                    for i in range(HT):
                        nc.tensor.matmul(
                            ps_q[:, i : i + 1], qrow[0:1, i * P : (i + 1) * P], ones1,
                            start=True, stop=True,
                        )
                    nc.vector.tensor_copy(qcols[b], ps_q)

                wqb_row = rows_pool.tile([1, O], dt, tag="row")
                nc.sync.dma_start(out=wqb_row, in_=wqb_in[None, :])
                wkb_row = rows_pool.tile([1, O], dt, tag="row")
                nc.sync.dma_start(out=wkb_row, in_=wkb_in[None, :])

                # biascol[b][:, j] = (Wq @ query_b + Wq_b + Wk_b)[o-tile j]
                ps_bias = [ps_mm.tile([P, OT], dt, tag="mm", name=f"ps_bias{b}")
                           for b in range(bpc)]
                for j in range(OT):
                    wq_nat = knat_pool.tile([P, H], dt, tag="knat")
                    nc.sync.dma_start(out=wq_nat, in_=wq_in[j * P : (j + 1) * P, :])
                    for i in range(HT):
                        ps_w = ps_tr.tile([P, P], dt, tag="tr")
                        nc.tensor.transpose(
                            ps_w, wq_nat[:, i * P : (i + 1) * P], ident_f
                        )
                        wqt_blk = wblk_pool.tile([P, P], dt, tag="wblk")
                        nc.vector.tensor_copy(wqt_blk, ps_w)
                        for b in range(bpc):
                            nc.tensor.matmul(
                                ps_bias[b][:, j : j + 1], wqt_blk,
                                qcols[b][:, i : i + 1],
                                start=(i == 0), stop=False,
                            )
                    for b in range(bpc):
                        nc.tensor.matmul(
                            ps_bias[b][:, j : j + 1],
                            wqb_row[0:1, j * P : (j + 1) * P], ones1,
                            start=False, stop=False,
                        )
                        nc.tensor.matmul(
                            ps_bias[b][:, j : j + 1],
                            wkb_row[0:1, j * P : (j + 1) * P], ones1,
                            start=False, stop=True,
                        )
                for b in range(bpc):
                    nc.vector.tensor_copy(biascol[b], ps_bias[b])

                # WkT resident: wkt[kk][p, :] = Wk[:, kk*128+p]
                wkt = [wkt_pool.tile([P, O], mdt, tag=f"wkt{kk}", name=f"wkt{kk}")
                       for kk in range(KT)]
                for j in range(OT):
                    wk_nat = knat_pool.tile([P, O], mdt, tag="knat")
                    kdma(out=wk_nat, in_=wk_in[j * P : (j + 1) * P, :])
                    for kk in range(KT):
                        ps_w = ps_tr.tile([P, P], mdt, tag="tr")
                        nc.tensor.transpose(
                            ps_w, wk_nat[:, kk * P : (kk + 1) * P], ident
                        )
                        nc.vector.tensor_copy(wkt[kk][:, j * P : (j + 1) * P], ps_w)

            # ---- main loop ----
            main_pools = [
                tc.tile_pool(name="tt", bufs=2),
                tc.tile_pool(name="prow", bufs=2),
                tc.tile_pool(name="attnp", bufs=1),
            ]
            if trans == "dve":
                main_pools += [
                    tc.tile_pool(name="perm", bufs=4),
                    tc.tile_pool(name="trf", bufs=1),
                    tc.tile_pool(name="ktrp", bufs=4),
                    tc.tile_pool(name="ps_ctxacc", bufs=1, space="PSUM"),
                ]
            elif trans == "pei":
                main_pools += [
                    tc.tile_pool(name="ktrp", bufs=2 * KT),
                    tc.tile_pool(name="ctxp", bufs=1),
                    tc.tile_pool(name="ps_tr2", bufs=2, space="PSUM"),
                    tc.tile_pool(name="ps_ctx", bufs=2, space="PSUM"),
                ]
            else:
                main_pools += [
                    tc.tile_pool(name="ktrp", bufs=KT),
                    tc.tile_pool(name="ctxp", bufs=1),
                    tc.tile_pool(name="ps_tr2", bufs=2, space="PSUM"),
                    tc.tile_pool(name="ps_ctx", bufs=2, space="PSUM"),
                ]
            import contextlib

            with contextlib.ExitStack() as stack:
                pools = [stack.enter_context(p) for p in main_pools]
                if trans == "dve":
                    tt_pool, prow_pool, attnp, perm_pool, trf_pool, ktr_pool, ps_ctxacc = pools
                elif trans == "pei":
                    tt_pool, prow_pool, attnp, ktr_pool, ctxp, ps_tr2, ps_ctx = pools
                else:
                    tt_pool, prow_pool, attnp, ktr_pool, ctxp, ps_tr2, ps_ctx = pools

                for _rep in range(repeat):
                  for b in range(bpc):
                    if trans == "dve":
                        ctxps = ps_ctxacc.tile([1, O], dt, tag="ctxacc")
                    else:
                        ctxacc = ctxp.tile([1, O], dt, tag="ctxacc")
                        nc.gpsimd.memset(ctxacc, 0.0)
                    attn_cols = attnp.tile([P, s_total // P], dt, tag=f"ac{b}")
                    denp = misc.tile([1, n_chunks], dt, tag=f"denp{b}")

                    def chunk_tail(c, knat, prow):
                        # p row -> columns (f32 for attn out, f32r for ctx),
                        # then ctx += p^T @ keys rows. Emitted one chunk late
                        # so the ACT exp hides under PE work.
                        ps_pt = ps_row.tile([P, NJ], dt, tag="row")
                        for jj in range(NJ):
                            nc.tensor.matmul(
                                ps_pt[:, jj : jj + 1],
                                prow[0:1, jj * P : (jj + 1) * P], ones1,
                                start=True, stop=True,
                            )
                        nc.vector.tensor_copy(
                            attn_cols[:, c * NJ : (c + 1) * NJ], ps_pt
                        )
                        pcol_r = prow_pool.tile([P, NJ], mdt, tag="pcol")
                        nc.vector.tensor_copy(pcol_r, ps_pt)
                        for kc in range(O // 512):
                            if trans == "dve":
                                tgt = ctxps[0:1, kc * 512 : (kc + 1) * 512]
                                for jj in range(NJ):
                                    nc.tensor.matmul(
                                        tgt, pcol_r[:, jj : jj + 1],
                                        knat[jj][:, kc * 512 : (kc + 1) * 512],
                                        start=(c == 0 and jj == 0),
                                        stop=(c == n_chunks - 1 and jj == NJ - 1),
                                    )
                            else:
                                ps_c = ps_ctx.tile([1, 512], dt, tag="ctx")
                                for jj in range(NJ):
                                    nc.tensor.matmul(
                                        ps_c, pcol_r[:, jj : jj + 1],
                                        knat[jj][:, kc * 512 : (kc + 1) * 512],
                                        start=(jj == 0), stop=(jj == NJ - 1),
                                    )
                                nc.vector.tensor_add(
                                    ctxacc[0:1, kc * 512 : (kc + 1) * 512],
                                    ctxacc[0:1, kc * 512 : (kc + 1) * 512],
                                    ps_c,
                                )

                    def load_chunk(c):
                        knat = []
                        for jj in range(NJ):
                            kn = knat_pool.tile([P, O], mdt, tag="knat", name="kn")
                            s0 = c * SC + jj * P
                            kdma(out=kn, in_=k_in[b, s0 : s0 + P, :])
                            knat.append(kn)
                        return knat

                    def trans_two(knat, kk):
                        # PE-transpose k-tiles kk, kk+1 of a loaded chunk
                        out = []
                        for k2 in (kk, kk + 1):
                            ps_t = ps_tr2.tile([P, SC], mdt, tag="tr")
                            for jj in range(NJ):
                                nc.tensor.transpose(
                                    ps_t[:, jj * P : (jj + 1) * P],
                                    knat[jj][:, k2 * P : (k2 + 1) * P],
                                    ident,
                                )
                            kt = ktr_pool.tile([P, SC], mdt, tag="ktr")
                            nc.vector.tensor_copy(kt, ps_t)
                            out.append(kt)
                        return out

                    if trans == "pei":
                        knat_cur = load_chunk(0)
                        ktr_cur = []
                        for kk in range(0, KT, 2):
                            ktr_cur += trans_two(knat_cur, kk)
                        pending = None
                        for c in range(n_chunks):
                            knat = knat_cur
                            ktr = ktr_cur
                            knat_next = (
                                load_chunk(c + 1) if c + 1 < n_chunks else None
                            )
                            ktr_next = []
                            ps_sc = ps_row.tile([1, SC], dt, tag="row")
                            tts = []
                            for m in range(OT):
                                ps_k = ps_mm.tile([P, SC], dt, tag="mm")
                                for kk in range(KT):
                                    nc.tensor.matmul(
                                        ps_k,
                                        wkt[kk][:, m * P : (m + 1) * P],
                                        ktr[kk],
                                        start=(kk == 0), stop=(kk == KT - 1),
                                    )
                                t_m = tt_pool.tile([P, SC], mdt, tag="tt",
                                                   name="t_m")
                                nc.scalar.activation(
                                    t_m, ps_k, AF.Tanh,
                                    bias=biascol[b][:, m : m + 1],
                                )
                                tts.append(t_m)
                                if m >= 1:
                                    nc.tensor.matmul(
                                        ps_sc, vcol[:, m - 1 : m], tts[m - 1],
                                        start=(m == 1), stop=False,
                                    )
                                if m == 2 and pending is not None:
                                    chunk_tail(*pending)
                                    pending = None
                                if m >= 8 and knat_next is not None:
                                    kk = (m - 8) * 2
                                    ktr_next += trans_two(knat_next, kk)
                            nc.tensor.matmul(
                                ps_sc, vcol[:, OT - 1 : OT], tts[OT - 1],
                                start=False, stop=True,
                            )
                            prow = prow_pool.tile([1, SC], dt, tag="prow")
                            nc.scalar.activation(
                                prow, ps_sc, AF.Exp, accum_out=denp[:, c : c + 1]
                            )
                            pending = (c, knat, prow)
                            knat_cur, ktr_cur = knat_next, ktr_next
                        if pending is not None:
                            chunk_tail(*pending)
                            pending = None
                    else:
                      pending = None
                      for c in range(n_chunks):
                        knat = []
                        for jj in range(NJ):
                            kn = knat_pool.tile([P, O], mdt, tag="knat", name="kn")
                            s0 = c * SC + jj * P
                            kdma(out=kn, in_=k_in[b, s0 : s0 + P, :])
                            knat.append(kn)

                        # keysT [k, s-chunk] tiles
                        if trans == "dve":
                            ktrq = []
                            for q in range(4):
                                ptile = perm_pool.tile([P, 1024], dt, tag="perm",
                                                       name="ptile")
                                tv = ptile.rearrange(
                                    "p (kk4 J aa) -> p kk4 J aa", kk4=4, aa=32
                                )
                                for I in range(4):
                                    for kk4 in range(4):
                                        eng = nc.sync if (I * 4 + kk4) % 2 == 0 else nc.scalar
                                        eng.dma_start(
                                            out=tv[32 * I : 32 * (I + 1), kk4],
                                            in_=kvs[b][c, q, I, :, kk4],
                                        )
                                trf = trf_pool.tile([P, 1024], dt, tag="trf",
                                                    name="trf")
                                nc.vector.transpose(trf, ptile)
                                kq = ktr_pool.tile([P, 1024], mdt, tag="ktr",
                                                   name="kq")
                                nc.vector.tensor_copy(kq, trf)
                                ktrq.append(kq)
                            ktr = [
                                ktrq[kk // 4][:, (kk % 4) * SC : (kk % 4 + 1) * SC]
                                for kk in range(KT)
                            ]
                        else:
                            ktr = []
                            for kk in range(KT):
                                ps_t = ps_tr2.tile([P, SC], mdt, tag="tr")
                                for jj in range(NJ):
                                    nc.tensor.transpose(
                                        ps_t[:, jj * P : (jj + 1) * P],
                                        knat[jj][:, kk * P : (kk + 1) * P],
                                        ident,
                                    )
                                kt = ktr_pool.tile([P, SC], mdt, tag="ktr")
                                nc.vector.tensor_copy(kt, ps_t)
                                ktr.append(kt)

                        if pending is not None:
                            chunk_tail(*pending)
                            pending = None

                        # k_projT [o, s-chunk] + tanh; matvec interleaved one
                        # o-tile behind so PE never waits on ACT.
                        ps_sc = ps_row.tile([1, SC], dt, tag="row")
                        tts = []
                        for m in range(OT):
                            ps_k = ps_mm.tile([P, SC], dt, tag="mm")
                            for kk in range(KT):
                                nc.tensor.matmul(
                                    ps_k, wkt[kk][:, m * P : (m + 1) * P], ktr[kk],
                                    start=(kk == 0), stop=(kk == KT - 1),
                                )
                            t_m = tt_pool.tile([P, SC], mdt, tag="tt", name="t_m")
                            nc.scalar.activation(
                                t_m, ps_k, AF.Tanh, bias=biascol[b][:, m : m + 1]
                            )
                            tts.append(t_m)
                            if m >= 1:
                                nc.tensor.matmul(
                                    ps_sc, vcol[:, m - 1 : m], tts[m - 1],
                                    start=(m == 1), stop=False,
                                )
                        nc.tensor.matmul(
                            ps_sc, vcol[:, OT - 1 : OT], tts[OT - 1],
                            start=False, stop=True,
                        )

                        # p = exp(scores); denominator partial
                        prow = prow_pool.tile([1, SC], dt, tag="prow")
                        nc.scalar.activation(
                            prow, ps_sc, AF.Exp, accum_out=denp[:, c : c + 1]
                        )
                        pending = (c, knat, prow)

                    if pending is not None:
                        chunk_tail(*pending)
                        pending = None

                    # ---- batch epilogue ----
                    den = misc.tile([1, 1], dt, tag=f"den{b}")
                    nc.vector.reduce_sum(den, denp, axis=mybir.AxisListType.X)
                    inv = misc.tile([1, 1], dt, tag=f"inv{b}")
                    nc.vector.reciprocal(inv, den)
                    if trans == "dve":
                        for kc in range(O // 512):
                            cst = prow_pool.tile([1, 512], dt, tag="ctxsb",
                                                 name="cst")
                            nc.vector.tensor_scalar_mul(
                                cst, ctxps[0:1, kc * 512 : (kc + 1) * 512], inv
                            )
                            nc.sync.dma_start(
                                out=ctx_out[b : b + 1, kc * 512 : (kc + 1) * 512],
                                in_=cst,
                            )
                    else:
                        nc.vector.tensor_scalar_mul(ctxacc, ctxacc, inv)
                        nc.sync.dma_start(out=ctx_out[b : b + 1, :], in_=ctxacc)

                    ps_ib = ps_row.tile([P, 1], dt, tag="row")
                    nc.tensor.matmul(ps_ib, ones_row, inv, start=True, stop=True)
                    invb = misc.tile([P, 1], dt, tag=f"invb{b}")
                    nc.vector.tensor_copy(invb, ps_ib)
                    nc.vector.tensor_scalar_mul(attn_cols, attn_cols, invb)
                    nc.sync.dma_start(out=attn_out[b], in_=attn_cols)

    nc.compile()
    return nc


def get_nc(bpc=BPC, s_total=S, mm_dtype=None, repeat=1, trans=None):
    if mm_dtype is None:
        mm_dtype = MM_DTYPE
    if trans is None:
        trans = TRANS
    key = (bpc, s_total, mm_dtype, repeat, trans)
    if key not in _CACHE:
        _CACHE[key] = _build(bpc, s_total, mm_dtype, repeat, trans)
    return _CACHE[key]


def _make_in_maps(inputs, bpc=BPC, s_total=S, ncores=NCORES):
    a = lambda x: np.ascontiguousarray(np.asarray(x, dtype=F32))
    query = a(inputs["query"]).reshape(-1, H)
    keys = a(inputs["keys"])
    wq, wqb = a(inputs["Wq_w"]), a(inputs["Wq_b"])
    wk, wkb = a(inputs["Wk_w"]), a(inputs["Wk_b"])
    vw = a(inputs["v_w"])
    in_maps = []
    for i in range(ncores):
        b0 = i * bpc
        in_maps.append(
            {
                "query": query[b0 : b0 + bpc],
                "keys": keys[b0 : b0 + bpc, :s_total],
                "wq": wq,
                "wqb": wqb,
                "wk": wk,
                "wkb": wkb,
                "vw": vw,
            }
        )
    return in_maps


def run(inputs, trace=False):
    from concourse.bass_utils import run_bass_kernel_spmd

    nc = get_nc()
    in_maps = _make_in_maps(inputs)
    res = run_bass_kernel_spmd(nc, in_maps, list(range(NCORES)), trace=trace)
    ctx = np.empty((B, 1, O), dtype=F32)
    attn = np.empty((B, S, 1), dtype=F32)
    for i in range(NCORES):
        r = res.results[i]
        for b in range(BPC):
            g = i * BPC + b
            ctx[g, 0, :] = r["ctx_out"][b]
            attn[g, :, 0] = r["attn_out"][b].T.reshape(-1)
    return (ctx, attn), res


def kernel(**inputs):
    (ctx, attn), _ = run(inputs, trace=False)
    return ctx, attn
